# revision 45
# baseline (speedup 1.0000x reference)
"""Trainium2 Bass kernel for nn_Polynomial: out = poly_basis(x) @ W.T + bias.

x: [500000, 8] f32.  basis = all 164 monomials of total degree 1..3 over the
8 features.  weight: [64, 164], bias: [64].

v3 strategy (pure data parallel over 8 cores, 62500 rows each, padded to
65536 = 8 supertiles x 8192 rows):
  - fp16 end-to-end on device (psum accumulates fp32); host converts.
  - rows-on-partitions basis build in SBUF with broadcast tensor_mul ops:
    chunk A [128 cols: x(8), pairs(36), triples k<=6 (84)], chunk B
    [64 cols: triples k=7 (36), const-1 (bias), zero pad].
  - the whole basis is transposed SBUF->SBUF by the DMA xbar
    (dma_start_transpose, 16x128 tiles): no PE transposes and no PSUM
    evacuation copies at all.  B's 64-col groups land pairwise on
    partitions [0:37] (even groups) and [64:101] (odd groups).
  - matmuls keep the small weight tables stationary (64-col ldweights) and
    stream the transposed basis as the moving operand at the max 512 free
    size; psum accumulates A + B per parity.
  - output is produced transposed ([64, rows]) and unpermuted on the host.
"""

import numpy as np

import concourse.bass as bass
import concourse.bacc as bacc
import concourse.mybir as mybir
from concourse import bass_utils
from concourse import tile

IN_F = 8
OUT_F = 64
KA = 128          # chunk-a columns
KB = 37           # chunk-b live columns (36 triples + const)
KB_PAD = 64       # chunk-b padded width
K_TOT = 165

G = 64            # row-groups per supertile
DEEP = 2          # pipeline buffer depth for the big SBUF pools
ROWS_PER_SUPER = 128 * G
N_CORES = 8
N_ROWS = 500000
ROWS_PER_CORE_RAW = N_ROWS // N_CORES  # 62500
ROWS_PER_CORE = 65536
N_SUPER = ROWS_PER_CORE // ROWS_PER_SUPER

F32 = mybir.dt.float32
F16 = mybir.dt.float16


def _pair_off(j: int) -> int:
    return j * (j + 1) // 2


def _trip_off(k: int) -> int:
    return k * (k + 1) * (k + 2) // 6


# Basis column layout (165 live columns in chunk order):
#   A[0..8)    x_i
#   A[8..44)   x_i * x_j      (i<=j), col = 8 + _pair_off(j) + i
#   A[44..128) x_i x_j x_k    (i<=j<=k<=6), col = 44 + _trip_off(k) + _pair_off(j) + i
#   B[0..36)   x_i x_j x_7    (i<=j<=7), col = _pair_off(j) + i
#   B[36]      1.0 (bias column)


def _term_col(e) -> int:
    """Column in the concatenated [A | B-live] order (0..164)."""
    facs = []
    for f in range(IN_F):
        facs += [f] * int(e[f])
    if len(facs) == 1:
        return facs[0]
    if len(facs) == 2:
        i, j = facs
        return 8 + _pair_off(j) + i
    i, j, k = facs
    if k <= 6:
        return 44 + _trip_off(k) + _pair_off(j) + i
    return KA + _pair_off(j) + i


def _exponents() -> np.ndarray:
    deg = np.arange(4)
    comb = np.array(np.meshgrid(*([deg] * IN_F))).T.reshape(-1, IN_F)
    s = comb.sum(axis=1)
    nz = (comb != 0).sum(axis=1)
    keep = ((nz == 1) & (s <= 3)) | ((nz > 1) & (s <= 3))
    return comb[keep].astype(np.int32)


def make_wtilde(weight: np.ndarray, bias: np.ndarray) -> np.ndarray:
    """Permute reference weight [64, 164] into W~ [165, 64] matching the
    on-chip basis column order; row 164 is the bias."""
    E = _exponents()
    wt = np.zeros((K_TOT, OUT_F), np.float32)
    for t in range(E.shape[0]):
        wt[_term_col(E[t])] += weight[:, t].astype(np.float32)
    wt[K_TOT - 1] = bias.astype(np.float32)
    return wt


def poly3_tile_kernel(tc, x_ap, w_ap, outT_ap, g: int = G, bench_reps=None,
                      stage: int = 3, deep: int = 2, sdeep: int = None):
    """x_ap: [rows, 8] f16, w_ap: [165, 64] f16, outT_ap: [64, rows] f16.
    rows must be a multiple of 128*g; g must be a multiple of 8.
    deep: buffer depth of the basis/output pools; sdeep: depth of the
    transposed-basis pool (decouples the xbar from the matmul tail)."""
    nc = tc.nc
    rows = x_ap.shape[0]
    assert rows % (128 * g) == 0 and g % 8 == 0
    n_super = rows // (128 * g)
    n_blk = g // 8
    sdeep = deep if sdeep is None else sdeep

    from contextlib import ExitStack

    with ExitStack() as ctx:
        cpool = ctx.enter_context(tc.tile_pool(name="cpool", bufs=1))
        xpool = ctx.enter_context(tc.tile_pool(name="xpool", bufs=deep + 1))
        bpool = ctx.enter_context(tc.tile_pool(name="bpool", bufs=deep))
        spool = ctx.enter_context(tc.tile_pool(name="spool", bufs=sdeep))
        opool = ctx.enter_context(tc.tile_pool(name="opool", bufs=deep))
        pso = ctx.enter_context(tc.tile_pool(name="pso", bufs=4, space="PSUM"))

        wa = cpool.tile([KA, OUT_F], F16)
        wb2 = cpool.tile([101, OUT_F], F16)
        nc.sync.dma_start(out=wa[:], in_=w_ap[0:KA])
        nc.sync.dma_start(out=wb2[0:KB, :], in_=w_ap[KA:K_TOT])
        nc.sync.dma_start(out=wb2[64:64 + KB, :], in_=w_ap[KA:K_TOT])

        xv = x_ap.rearrange("(t p g) f -> t p g f", p=128, g=g)
        otv = outT_ap.rearrange("o (t m) -> t o m", m=128 * g)

        def do_supertile(t):
            x3 = xpool.tile([128, g, IN_F], F16, tag="x3")
            # queue discipline: each pipeline stage gets its own issue queue so
            # FIFO order never chains iteration t+1 behind the tail of t.
            # in: gpsimd SWDGE; xbar transposes: sync; out: scalar; evacuation
            # copies: ACT only (DVE does only basis builds).
            nc.gpsimd.dma_start(out=x3[:], in_=xv[t])

            sbO = opool.tile([OUT_F, 128 * g], F16, tag="sbO")

            if stage == 0:
                nc.vector.tensor_copy(out=sbO[:, 0:g * IN_F],
                                      in_=x3[0:OUT_F, :, :])
                nc.scalar.dma_start(out=otv[t], in_=sbO[:])
                return

            bA = bpool.tile([128, g, KA], F16, tag="bA")
            bB = bpool.tile([128, g, KB_PAD], F16, tag="bB")
            # all products on DVE (gpsimd's software multiply is far below
            # roofline); gpsimd keeps only the cheap memsets + in-DMA issue;
            # the x-column copy rides ACT to shave the top engine (DVE)
            nc.scalar.copy(out=bA[:, :, 0:IN_F], in_=x3[:])
            nc.gpsimd.memset(bB[:, :, 36:37], 1.0)
            nc.gpsimd.memset(bB[:, :, 37:KB_PAD], 0.0)
            for j in range(IN_F):
                w_ = j + 1
                o = 8 + _pair_off(j)
                nc.vector.tensor_mul(
                    out=bA[:, :, o : o + w_],
                    in0=x3[:, :, 0:w_],
                    in1=x3[:, :, j : j + 1].broadcast_to([128, g, w_]),
                )
            for k in range(7):
                w_ = _pair_off(k + 1)
                o = 44 + _trip_off(k)
                nc.vector.tensor_mul(
                    out=bA[:, :, o : o + w_],
                    in0=bA[:, :, 8 : 8 + w_],
                    in1=x3[:, :, k : k + 1].broadcast_to([128, g, w_]),
                )
            nc.vector.tensor_mul(
                out=bB[:, :, 0:36],
                in0=bA[:, :, 8:44],
                in1=x3[:, :, 7:8].broadcast_to([128, g, 36]),
            )

            if stage == 1:
                nc.vector.tensor_copy(out=sbO[:], in_=bA[0:OUT_F, :, 0:128])
                nc.scalar.dma_start(out=otv[t], in_=sbO[:])
                return

            # xbar transposes: sbAT[c, gi, r] = bA[r, gi, c];
            # sbBT[(gi%2)*64 + c, gi//2, r] = bB[r, gi, c]
            sbAT = spool.tile([128, g, 128], F16, tag="sbAT")
            sbBT = spool.tile([128, g // 2, 128], F16, tag="sbBT")
            # both on the sync queue: concurrent xbar transposes on different
            # HWDGE engines corrupt each other (shared xbar resource)
            nc.sync.dma_start_transpose(sbAT[:], bA[:])
            nc.sync.dma_start_transpose(sbBT[:], bB[:])

            if stage == 2:
                nc.vector.tensor_copy(out=sbO[:], in_=sbAT[0:OUT_F, :, 0:128])
                nc.scalar.dma_start(out=otv[t], in_=sbO[:])
                return

            for blk in range(n_blk):
                g0 = blk * 8
                # one 2-bank psum tile per block: half 0 = even groups
                # (g0, g0+2, ...), half 1 = odd; single-copy evacuation
                psO = pso.tile([OUT_F, 2, 4, 128], F32, tag="psO")
                nc.tensor.matmul(psO[:, 0], lhsT=wa[:],
                                 rhs=sbAT[:, g0 : g0 + 8 : 2, :],
                                 start=True, stop=False)
                nc.tensor.matmul(psO[:, 0], lhsT=wb2[0:KB, :],
                                 rhs=sbBT[0:KB, 4 * blk : 4 * blk + 4, :],
                                 start=False, stop=True)
                nc.tensor.matmul(psO[:, 1], lhsT=wa[:],
                                 rhs=sbAT[:, g0 + 1 : g0 + 8 : 2, :],
                                 start=True, stop=False)
                nc.tensor.matmul(psO[:, 1], lhsT=wb2[64:64 + KB, :],
                                 rhs=sbBT[64:64 + KB, 4 * blk : 4 * blk + 4, :],
                                 start=False, stop=True)
                sbOv = sbO[:, blk * 1024 : (blk + 1) * 1024].rearrange(
                    "o (gp eo r) -> o eo gp r", eo=2, r=128)
                nc.scalar.copy(out=sbOv[:], in_=psO[:])

            nc.scalar.dma_start(out=otv[t], in_=sbO[:])

        if bench_reps is None:
            for t in range(n_super):
                do_supertile(t)
        elif isinstance(bench_reps, tuple):
            # unrolled benchmark: repeat one aliased supertile n times
            for _ in range(bench_reps[0]):
                do_supertile(0)
        else:
            with tc.For_i(0, bench_reps, 1):
                do_supertile(0)


_CACHED_NC = {}


def build_nc(rows_per_core: int = ROWS_PER_CORE, g: int = G, deep: int = DEEP):
    key = (rows_per_core, g, deep)
    if key not in _CACHED_NC:
        nc = bacc.Bacc("TRN2", target_bir_lowering=False, debug=False, num_devices=N_CORES)
        x_d = nc.dram_tensor("x", [rows_per_core, IN_F], F16, kind="ExternalInput")
        w_d = nc.dram_tensor("wt", [K_TOT, OUT_F], F16, kind="ExternalInput")
        o_d = nc.dram_tensor("outT", [OUT_F, rows_per_core], F16, kind="ExternalOutput")
        with tile.TileContext(nc) as tc:
            poly3_tile_kernel(tc, x_d.ap(), w_d.ap(), o_d.ap(), g=g, deep=deep)
        nc.compile()
        _CACHED_NC[key] = nc
    return _CACHED_NC[key]


def build_nc_reps(reps: int, g: int = G, stage: int = 3):
    key = ("reps", reps, g, stage)
    if key not in _CACHED_NC:
        rows = 128 * g
        nc = bacc.Bacc("TRN2", target_bir_lowering=False, debug=False, num_devices=N_CORES)
        x_d = nc.dram_tensor("x", [rows, IN_F], F16, kind="ExternalInput")
        w_d = nc.dram_tensor("wt", [K_TOT, OUT_F], F16, kind="ExternalInput")
        o_d = nc.dram_tensor("outT", [OUT_F, rows], F16, kind="ExternalOutput")
        with tile.TileContext(nc) as tc:
            poly3_tile_kernel(tc, x_d.ap(), w_d.ap(), o_d.ap(), g=g,
                              bench_reps=reps, stage=stage)
        nc.compile()
        _CACHED_NC[key] = nc
    return _CACHED_NC[key]


def build_nc_unrolled(n: int, g: int = G, stage: int = 3, deep: int = None):
    deep = DEEP if deep is None else deep
    key = ("unroll", n, g, stage, deep)
    if key not in _CACHED_NC:
        rows = 128 * g
        nc = bacc.Bacc("TRN2", target_bir_lowering=False, debug=False, num_devices=N_CORES)
        x_d = nc.dram_tensor("x", [rows, IN_F], F16, kind="ExternalInput")
        w_d = nc.dram_tensor("wt", [K_TOT, OUT_F], F16, kind="ExternalInput")
        o_d = nc.dram_tensor("outT", [OUT_F, rows], F16, kind="ExternalOutput")
        with tile.TileContext(nc) as tc:
            poly3_tile_kernel(tc, x_d.ap(), w_d.ap(), o_d.ap(), g=g,
                              bench_reps=(n,), stage=stage, deep=deep)
        nc.compile()
        _CACHED_NC[key] = nc
    return _CACHED_NC[key]


def unpermute_outT(outT: np.ndarray, g: int = G) -> np.ndarray:
    """outT [64, rows] fp16 with free order (t, gi, r) -> rows-major
    [rows, 64] fp32; row n = t*128*g + r*g + gi."""
    n_super = outT.shape[1] // (128 * g)
    v = outT.reshape(OUT_F, n_super, g, 128)
    return np.ascontiguousarray(
        v.transpose(1, 3, 2, 0).reshape(n_super * 128 * g, OUT_F),
        ).astype(np.float32)


def kernel(x, weight, bias):
    x = np.asarray(x, dtype=np.float32)
    wt = make_wtilde(np.asarray(weight, dtype=np.float32),
                     np.asarray(bias, dtype=np.float32)).astype(np.float16)
    nc = build_nc()

    in_maps = []
    for c in range(N_CORES):
        shard = x[c * ROWS_PER_CORE_RAW : (c + 1) * ROWS_PER_CORE_RAW]
        xpad = np.zeros((ROWS_PER_CORE, IN_F), np.float16)
        xpad[: shard.shape[0]] = shard.astype(np.float16)
        in_maps.append({"x": xpad, "wt": wt})

    res = bass_utils.run_bass_kernel_spmd(nc, in_maps, core_ids=list(range(N_CORES)))
    outs = [unpermute_outT(r["outT"])[:ROWS_PER_CORE_RAW] for r in res.results]
    return np.concatenate(outs, axis=0)
